# revision 1
# baseline (speedup 1.0000x reference)
"""TRN2 Bass kernel for the ConceptualMambaBlock problem (bf16 pipeline, v3).

Math (reference):
    x: [B=4, T=96, N=512, H=128] f32
    expanded = x @ W_exp.T + b_exp            # [B,T,N,2H]
    primary, gating = split(expanded, 2, -1)
    s_t = 0.9*s_{t-1} + 0.1*gating_t          # EMA along T
    out = (primary * sigmoid(s)) @ W_con.T + b_con

Strategy:
  - Shard (B x N/2) over 8 cores: core c -> batch c//2, node half c%2.
  - Host pre-transposes each core's x shard to [H, NLOC*T] (t fastest) and
    casts to bf16; outputs come back bf16 and are upcast on host.
  - 24 pairs of 1024 columns per core; matmuls and PSUM tiles are 512 wide
    (PSUM bank limit), SBUF elementwise ops are 1024 wide where possible.
  - Sigma-shift for the gating bias: scan sigma = s - b_g needs -0.9*b_g
    added at each node's t=0 column of the gating PSUM.  This rides the PE
    as a tiny accumulate-matmul (lhsT = [1,H] row of -0.9*b_g, rhs = ones)
    closing the mm1g accumulation group; +b_g is folded into the sigmoid
    bias port.  All other biases ride hardware bias ports too.
  - EMA via DVE tensor_tensor_scan per 512 block (mask 0.0 at node starts),
    chained across blocks through the per-partition `initial` port.
  - Gate-multiply y=(pp+b1p)*sigma alternates between DVE stt (bias via
    scalar port) and ACT-copy(+bias)+Pool-multiply to balance engine load.
  - Deep software pipeline with per-engine streams ordered oldest-first:
      step s:  PE : mm2(s-4), mm1g+fixup(s), mm1p(s-1)
               ACT: outcopy(s-5), ppcopy(s-2), sig(s-2)
               DVE: stt(s-3), scan(s-1)
               Pool: mult(s-3), output DMA triggers
    PSUM: pg[1 bank]x3 + pp[1]x3 + po[2 banks]x1 = 8 banks.
    NOTE: this schedule is a sensitive local optimum (~104.5us): reordering
    DVE (scan first), adding SBUF pool bufs, or tightening the lags all
    measured 6-20%% WORSE on HW.  Measure 3+ runs before accepting changes;
    single runs vary up to 20%%.
"""

import numpy as np
import ml_dtypes

import concourse.bacc as bacc
import concourse.bass as bass  # noqa: F401
import concourse.mybir as mybir
import concourse.tile as tile
from concourse.bass_utils import run_bass_kernel_spmd

F32 = mybir.dt.float32
BF16 = mybir.dt.bfloat16
AF = mybir.ActivationFunctionType
ALU = mybir.AluOpType

B, T, N, H = 4, 96, 512, 128
NCORES = 8
NLOC = N // 2            # 256 nodes per core
TOK = NLOC * T           # 24576 columns per core
P = 1024                 # pair width
HP = 512                 # half pair
NPAIR = TOK // P         # 24
GRP = 4                  # pairs per DMA group
NGRP = NPAIR // GRP      # 6

# gate-multiply on DVE (True) vs ACT-copy+Pool (False); ~0.6 on DVE
MULT_ON_DVE = [(g % 3 == 2 and g != 17) or g >= NPAIR - 3
               for g in range(NPAIR)]

_NC_CACHE = None


def _fixup_cols(g):
    """(offset, count) pairs of the strided t0 views in each 512 half."""
    j0 = (-P * g) % 96
    a = (j0, len(range(j0, HP, 96)))
    first_b = j0 + 96 * ((HP - j0 + 95) // 96) - HP
    b = (first_b, len(range(first_b, HP, 96)))
    return a, b


def _build():
    nc = bacc.Bacc()

    xt_h = nc.dram_tensor("xt", [H, NPAIR, P], BF16, kind="ExternalInput")
    bnegrow_h = nc.dram_tensor("bnegrow", [1, H], BF16, kind="ExternalInput")
    wpack_h = nc.dram_tensor("wpack", [H, 3 * H], BF16, kind="ExternalInput")
    bpack_h = nc.dram_tensor("bpack", [H, 4], F32, kind="ExternalInput")
    mask_h = nc.dram_tensor("mask", [H, 3, P], BF16, kind="ExternalInput")
    out_h = nc.dram_tensor("out", [H, NPAIR, P], BF16, kind="ExternalOutput")

    with tile.TileContext(nc) as tc:
        with (
            tc.tile_pool(name="consts", bufs=1) as cp,
            tc.tile_pool(name="io", bufs=2) as io,
            tc.tile_pool(name="mid", bufs=3) as mid,
            tc.tile_pool(name="ps", bufs=1, space="PSUM") as ps,
        ):
            wpack_sb = cp.tile([H, 3 * H], BF16, tag="wpack")
            nc.sync.dma_start(out=wpack_sb[:], in_=wpack_h[:, :])
            bnegrow_sb = cp.tile([1, H], BF16, tag="bnegrow")
            nc.sync.dma_start(out=bnegrow_sb[:], in_=bnegrow_h[:, :])
            # first input pair right behind the weights: it gates the pipeline
            xt0 = io.tile([H, GRP, P], BF16, tag="xt", name="xt0")
            nc.sync.dma_start(out=xt0[:, 0, :], in_=xt_h[:, 0, :])
            bpack_sb = cp.tile([H, 4], F32, tag="bpack")
            nc.scalar.dma_start(out=bpack_sb[:], in_=bpack_h[:, :])
            mask_sb = cp.tile([H, 3, P], BF16, tag="mask")
            nc.sync.dma_start(out=mask_sb[:, 0, :], in_=mask_h[:, 0, :])
            nc.scalar.dma_start(out=mask_sb[:, 1:3, :], in_=mask_h[:, 1:3, :])

            ones_sb = cp.tile([1, P], BF16, tag="ones")
            nc.gpsimd.memset(ones_sb[:], 1.0)

            # warm the ACT function tables before the pipeline needs them
            warm = cp.tile([H, 1], BF16, tag="warm")
            nc.scalar.activation(warm[:], wpack_sb[:, 0:1], AF.Sigmoid,
                                 bias=0.0, scale=1.0)
            nc.scalar.activation(warm[:], wpack_sb[:, 0:1], AF.Identity,
                                 bias=0.0, scale=1.0)

            w1g = wpack_sb[:, 0:H]
            w1p = wpack_sb[:, H : 2 * H]
            w2 = wpack_sb[:, 2 * H : 3 * H]
            bneg = bpack_sb[:, 0:1]   # -0.9*b_g
            bg = bpack_sb[:, 1:2]     # b_g
            b1p = bpack_sb[:, 2:3]    # b_exp[:H]
            b2 = bpack_sb[:, 3:4]     # b_con

            state = {}
            carry = {}
            xg4 = {}
            ob4 = {}

            def emit_load(gi):
                if gi == 0:
                    nc.sync.dma_start(
                        out=xt0[:, 1:4, :], in_=xt_h[:, 1:4, :]
                    )
                    xg4[0] = xt0
                    return
                t = io.tile([H, GRP, P], BF16, tag="xt", name=f"xt{gi}")
                nc.sync.dma_start(
                    out=t[:, 0:2, :], in_=xt_h[:, gi * GRP : gi * GRP + 2, :]
                )
                nc.sync.dma_start(
                    out=t[:, 2:4, :],
                    in_=xt_h[:, gi * GRP + 2 : (gi + 1) * GRP, :]
                )
                xg4[gi] = t

            def emit_mm1g(g):
                xg = xg4[g // GRP][:, g % GRP, :]
                (oa, na), (ob_, nb) = _fixup_cols(g)
                pga = ps.tile([H, HP], F32, tag="pg", name=f"pga{g}", bufs=3)
                pgb = ps.tile([H, HP], F32, tag="pg", name=f"pgb{g}", bufs=3)
                nc.tensor.matmul(pga[:], lhsT=w1g, rhs=xg[:, 0:HP],
                                 start=True, stop=False)
                nc.tensor.matmul(pgb[:], lhsT=w1g, rhs=xg[:, HP:P],
                                 start=True, stop=False)
                # sigma-shift: accumulate -0.9*b_g onto each node's t0 column
                nc.tensor.matmul(pga[:, oa :: 96], lhsT=bnegrow_sb[:],
                                 rhs=ones_sb[:, 0:na], start=False, stop=True)
                nc.tensor.matmul(pgb[:, ob_ :: 96], lhsT=bnegrow_sb[:],
                                 rhs=ones_sb[:, 0:nb], start=False, stop=True)
                state[g] = {"pg": (pga, pgb)}

            def emit_scan(g):
                pga, pgb = state[g]["pg"]
                s = mid.tile([H, P], BF16, tag="s", name=f"s{g}")
                init = 0.0 if g == 0 else carry.pop(g - 1)
                m = mask_sb[:, g % 3, :]
                nc.vector.tensor_tensor_scan(
                    out=s[:, 0:HP], data0=m[:, 0:HP], data1=pga[:],
                    initial=init, op0=ALU.mult, op1=ALU.add,
                )
                nc.vector.tensor_tensor_scan(
                    out=s[:, HP:P], data0=m[:, HP:P], data1=pgb[:],
                    initial=s[:, HP - 1 : HP], op0=ALU.mult, op1=ALU.add,
                )
                carry[g] = s[:, P - 1 : P]
                state[g]["s"] = s

            def emit_sig(g):
                s = state[g]["s"]
                sg = mid.tile([H, P], BF16, tag="sg", name=f"sg{g}")
                nc.scalar.activation(sg[:], s[:], AF.Sigmoid, bias=bg, scale=1.0)
                state[g]["sig"] = sg

            def emit_mm1p(g):
                xg = xg4[g // GRP][:, g % GRP, :]
                ppa = ps.tile([H, HP], F32, tag="pp", name=f"ppa{g}", bufs=3)
                ppb = ps.tile([H, HP], F32, tag="pp", name=f"ppb{g}", bufs=3)
                nc.tensor.matmul(ppa[:], lhsT=w1p, rhs=xg[:, 0:HP],
                                 start=True, stop=True)
                nc.tensor.matmul(ppb[:], lhsT=w1p, rhs=xg[:, HP:P],
                                 start=True, stop=True)
                state[g]["pp"] = (ppa, ppb)

            def emit_ppcopy(g):
                ppa, ppb = state[g]["pp"]
                pps = mid.tile([H, P], BF16, tag="pps", name=f"pps{g}")
                nc.scalar.activation(pps[:, 0:HP], ppa[:], AF.Identity,
                                     bias=b1p, scale=1.0)
                nc.scalar.activation(pps[:, HP:P], ppb[:], AF.Identity,
                                     bias=b1p, scale=1.0)
                state[g]["pps"] = pps

            def emit_mult(g):
                sg = state[g]["sig"]
                y = mid.tile([H, P], BF16, tag="y", name=f"y{g}")
                if MULT_ON_DVE[g]:
                    ppa, ppb = state[g]["pp"]
                    nc.vector.scalar_tensor_tensor(
                        out=y[:, 0:HP], in0=ppa[:], scalar=b1p, in1=sg[:, 0:HP],
                        op0=ALU.add, op1=ALU.mult,
                    )
                    nc.vector.scalar_tensor_tensor(
                        out=y[:, HP:P], in0=ppb[:], scalar=b1p, in1=sg[:, HP:P],
                        op0=ALU.add, op1=ALU.mult,
                    )
                else:
                    pps = state[g]["pps"]
                    nc.gpsimd.tensor_tensor(out=y[:], in0=pps[:], in1=sg[:],
                                            op=ALU.mult)
                state[g]["y"] = y

            def emit_mm2(g):
                y = state[g]["y"]
                po = ps.tile([H, P], F32, tag="po", name=f"po{g}", bufs=1)
                nc.tensor.matmul(po[:, 0:HP], lhsT=w2, rhs=y[:, 0:HP],
                                 start=True, stop=True)
                nc.tensor.matmul(po[:, HP:P], lhsT=w2, rhs=y[:, HP:P],
                                 start=True, stop=True)
                state[g]["po"] = po

            def emit_out(g):
                po = state[g]["po"]
                gi = g // GRP
                if g % GRP == 0:
                    ob4[gi] = io.tile([H, GRP, P], BF16, tag="ob", name=f"ob{gi}")
                dst = ob4[gi][:, g % GRP, :]
                nc.scalar.activation(dst[:], po[:], AF.Identity,
                                     bias=b2, scale=1.0)
                if gi == NGRP - 1:
                    # drain the last group piecewise so the final DMA is small
                    k = g % GRP
                    eng = [nc.gpsimd, nc.sync, nc.gpsimd, nc.sync][k]
                    eng.dma_start(
                        out=out_h[:, gi * GRP + k, :],
                        in_=ob4[gi][:, k, :],
                    )
                    if k == GRP - 1:
                        ob4.pop(gi)
                elif g % GRP == GRP - 1:
                    nc.gpsimd.dma_start(
                        out=out_h[:, gi * GRP : (gi + 1) * GRP, :],
                        in_=ob4.pop(gi)[:],
                    )
                del state[g]

            # prefetch first two groups
            emit_load(0)
            emit_load(1)

            STEPS = NPAIR + 5
            for s in range(STEPS):
                if s % GRP == 0 and s // GRP + 2 < NGRP:
                    emit_load(s // GRP + 2)

                g_out = s - 5
                g_mm2 = s - 4
                g_mult = s - 3
                g_sig = s - 2
                g_mm1p = s - 1
                g_new = s

                # PE: oldest first
                if 0 <= g_mm2 < NPAIR:
                    emit_mm2(g_mm2)
                if g_new < NPAIR:
                    emit_mm1g(g_new)
                if 0 <= g_mm1p < NPAIR:
                    emit_mm1p(g_mm1p)
                # ACT: oldest first
                if 0 <= g_out < NPAIR:
                    emit_out(g_out)
                if 0 <= g_sig < NPAIR:
                    if not MULT_ON_DVE[g_sig]:
                        emit_ppcopy(g_sig)
                    emit_sig(g_sig)
                # DVE: stt (deps 1+ step old) then scan
                if 0 <= g_mult < NPAIR:
                    emit_mult(g_mult)
                if 0 <= g_mm1p < NPAIR:
                    emit_scan(g_mm1p)

    nc.finalize()
    return nc


def _get_nc():
    global _NC_CACHE
    if _NC_CACHE is None:
        _NC_CACHE = _build()
    return _NC_CACHE


def _in_maps(x, W_exp, b_exp, W_con, b_con):
    wpack = np.concatenate(
        [(0.1 * W_exp[H:, :]).T, W_exp[:H, :].T, W_con.T], axis=1
    ).astype(ml_dtypes.bfloat16)
    wpack = np.ascontiguousarray(wpack)
    bg = b_exp[H:]
    bpack = np.stack(
        [-0.9 * bg, bg, b_exp[:H], b_con], axis=1
    ).astype(np.float32)
    bpack = np.ascontiguousarray(bpack)

    mask = np.full((H, 3, P), 0.9, np.float32)
    for ph in range(3):
        o = (-P * ph) % 96
        mask[:, ph, o::96] = 0.0
    mask = np.ascontiguousarray(mask.astype(ml_dtypes.bfloat16))

    maps = []
    for c in range(NCORES):
        bb, nh = c // 2, c % 2
        xs = x[bb, :, nh * NLOC : (nh + 1) * NLOC, :]  # [T, NLOC, H]
        xT = np.ascontiguousarray(xs.transpose(2, 1, 0)).reshape(H, NPAIR, P)
        maps.append(
            {
                "xt": xT.astype(ml_dtypes.bfloat16),
                "wpack": wpack,
                "bpack": bpack,
                "mask": mask,
                "bnegrow": np.ascontiguousarray(
                    (-0.9 * bg)[None, :].astype(ml_dtypes.bfloat16)
                ),
            }
        )
    return maps


def run_spmd(x, W_exp, b_exp, W_con, b_con, **spmd_kwargs):
    """Run the 8-core kernel; returns (full_output, BassKernelResults)."""
    maps = _in_maps(x, W_exp, b_exp, W_con, b_con)
    res = run_bass_kernel_spmd(
        _get_nc(), maps, core_ids=list(range(NCORES)), **spmd_kwargs
    )
    out = np.empty((B, T, N, H), dtype=np.float32)
    for c in range(NCORES):
        bb, nh = c // 2, c % 2
        oT = np.asarray(res.results[c]["out"]).astype(np.float32)
        oT = oT.reshape(H, NLOC, T)
        out[bb, :, nh * NLOC : (nh + 1) * NLOC, :] = oT.transpose(2, 1, 0)
    return out, res


def kernel(spatial_temporal_representation, W_exp, b_exp, W_con, b_con):
    out, _ = run_spmd(
        np.asarray(spatial_temporal_representation, dtype=np.float32),
        np.asarray(W_exp, dtype=np.float32),
        np.asarray(b_exp, dtype=np.float32),
        np.asarray(W_con, dtype=np.float32),
        np.asarray(b_con, dtype=np.float32),
    )
    return out

